# revision 1
# baseline (speedup 1.0000x reference)
"""Trainium2 Bass kernel for nn_ContextualLSTMCell_74955769250717.

The reference multiplies the low-rank context perturbations A_i/A_h by 0.0
(faithful to the original model), so the required math reduces *exactly*
(bitwise: 0.0*finite + W == W) to a plain LSTM cell:

    ifgo  = x @ Wi + Wi_b + h @ Wh + Wh_b            [B, 4H]
    i,f,g,o = gates(ifgo);  c_new = f*c + i*g;  h_new = o*tanh(c_new)

Sharding: tensor-parallel over the 4H gate dimension across 8 cores. Core k
owns hidden slice k*128:(k+1)*128 of every gate (512 of the 4096 gate
columns, reordered [i|f|o|g] so one Sigmoid activation covers 3 gates),
computes its ifgo columns with K-tile matmuls (K = E+H = 2048 combined,
batch as the stationary operand so each matmul streams N=512 columns), adds
the bias via a rank-1 ones-matmul, and finishes the cell elementwise for its
hidden slice. The host concatenates the 8 independent slices.

Raw Bass (no Tile): this toolchain enforces ONE sync-wait slot per
instruction, which Tile's auto-emitted kernel-tail drain violates; with
explicit semaphores every multi-producer join is a chain of standalone
single-wait instructions. Weights stream in small chunked DMAs (HWDGE DMAs
from one engine execute FIFO, so chunk k completes before chunk k+1 and one
cumulative semaphore tracks them) so the matmul stream starts early and
overlaps the remaining DMA traffic; the kernel is DMA-bound.

Precision modes (PREC):
  'f32'    exact fp32 matmul (4 cycles/row on PE)
  'f32r'   fp32 data, PE float32r mode (full rate at N=512)
  'bf16'   bf16 weights+activations (~1e-3 rel err)
  'bf16x3' hi/lo-split bf16, 3-pass compensated matmul (~1e-5 rel err)
"""

from contextlib import ExitStack

import ml_dtypes
import numpy as np

import concourse.bass as bass
import concourse.mybir as mybir
from concourse.bass_utils import run_bass_kernel_spmd

B, E, H = 16, 1024, 1024
H4 = 4 * H
K = E + H              # combined contraction dim (x and h stacked)
KT = K // 128          # 16 K-tiles of 128
N_CORES = 8
NSL = H4 // N_CORES    # 512 gate columns per core
HS = H // N_CORES      # 128 hidden units per core

PREC = 'bf16x3'

AF = mybir.ActivationFunctionType

_WDT = {
    'f32': mybir.dt.float32,
    'f32r': mybir.dt.float32r,
    'bf16': mybir.dt.bfloat16,
    'bf16x3': mybir.dt.bfloat16,
}

_built = {}
_CHUNKS_OVERRIDE = None


def _build(prec):
    """Build the (single-core, SPMD-replicated) raw-bass program."""
    wdt = _WDT[prec]
    x3 = prec == 'bf16x3'
    n_w = 2 * KT if x3 else KT      # weight k-tiles in DRAM (hi+lo)
    nbias = 2 if x3 else 1
    un = 3 * B if x3 else B         # stationary u cols/k-tile (hi|pad|lo)
    ext = B + NSL                   # ones + bias block (bias hi/lo on rows 0/1)
    MM = 3 * B if x3 else B         # matmul output partitions (hi|pad|lo)
    # Uneven W chunks: first chunk big enough that once PE starts it never
    # outruns the FIFO DMA stream (supply 364ns/k-tile bf16, 728 fp32 vs
    # one 213ns stream per k-tile); trailing chunks small to cut the tail.
    chunks = (_CHUNKS_OVERRIDE or {
        'bf16x3': [2] * 15 + [1, 1],
        'bf16':   [1, 1, 2, 2, 2, 2, 2, 2, 1, 1],
        'f32r':   [1] * 16,
        'f32':    [1] * 16,
    })[prec]
    assert sum(chunks) == n_w
    offs = [sum(chunks[:j]) for j in range(len(chunks))]   # k-tile offsets

    nc = bass.Bass()
    ub_d = nc.dram_tensor("ub", [128, KT * un + ext], wdt, kind="ExternalInput")
    w_d = nc.dram_tensor("w", [128, n_w * NSL], wdt, kind="ExternalInput")
    c_d = nc.dram_tensor("c", [B, HS], mybir.dt.float32, kind="ExternalInput")
    hc_d = nc.dram_tensor("hc", [B, 2 * HS], mybir.dt.float32,
                          kind="ExternalOutput")

    with ExitStack() as ctx:
        e = ctx.enter_context
        f32 = mybir.dt.float32
        sb_ub = e(nc.sbuf_tensor("sb_ub", [128, KT * un + ext], wdt))
        w_sb = [e(nc.sbuf_tensor(f"w_sb{j}", [128, cpt * NSL], wdt))
                for j, cpt in enumerate(chunks)]
        sb_c = e(nc.sbuf_tensor("c_sb", [B, HS], f32))
        ifgo = e(nc.sbuf_tensor("ifgo", [B, NSL], f32))
        tmpC = e(nc.sbuf_tensor("tmpC", [B, NSL], f32))
        gates = e(nc.sbuf_tensor("gates", [B, NSL], f32))
        fc = e(nc.sbuf_tensor("fc", [B, HS], f32))
        ig = e(nc.sbuf_tensor("ig", [B, HS], f32))
        tnh = e(nc.sbuf_tensor("tnh", [B, HS], f32))
        hc = e(nc.sbuf_tensor("hc_sb", [B, 2 * HS], f32))
        ps = e(nc.psum_tensor("ps", [MM, NSL], f32))

        s_ub = e(nc.semaphore("s_ub"))
        s_c = e(nc.semaphore("s_c"))
        s_w = [e(nc.semaphore(f"s_w{j}")) for j in range(len(chunks))]
        s_hi = e(nc.semaphore("s_hi"))
        s_mm = e(nc.semaphore("s_mm"))
        s_act = e(nc.semaphore("s_act"))
        s_dve = e(nc.semaphore("s_dve"))
        s_done = e(nc.semaphore("s_done"))
        s_out = e(nc.semaphore("s_out"))

        # x3 stationary layout per k-tile: [u_hi | zero pad | u_lo] (3B
        # cols; the pad puts u_lo's output rows at partition 32, the PSUM
        # partition-base alignment). One W_hi stream computes u_hi@W_hi
        # (psum rows 0:B) AND u_lo@W_hi (rows 2B:3B) in a single matmul —
        # cost is per streamed column, independent of M. The W_lo pass uses
        # just the u_hi half and accumulates straight onto rows 0:B.
        ones = sb_ub[0:nbias, KT * un:KT * un + B]
        bias = sb_ub[0:nbias, KT * un + B:KT * un + B + NSL]

        # (lhsT, rhs, out) per chunk; stream order = w k-tile order.
        plan = [[] for _ in chunks]
        for j, (cpt, off) in enumerate(zip(chunks, offs)):
            for tt in range(cpt):
                kt = off + tt                 # global w k-tile index
                rh = w_sb[j][:, tt * NSL:(tt + 1) * NSL]
                if x3 and kt >= KT:           # lo half: u_hi vs W_lo
                    lh = sb_ub[:, (kt - KT) * un:(kt - KT) * un + B]
                    plan[j].append((lh, rh, ps[0:B, :]))
                else:                         # [u_hi|u_lo] vs W_hi
                    lh = sb_ub[:, kt * un:kt * un + un]
                    plan[j].append((lh, rh, ps[0:MM, :]))

        with nc.Block() as block:

            @block.sync
            def _(sync):
                # ub first (PE blocks on it), then the W chunk stream;
                # per-chunk semaphores: the 16 per-engine +1 increments of
                # consecutive DMAs interleave, so one cumulative semaphore
                # would be racy.
                for j, (cpt, off) in enumerate(zip(chunks, offs)):
                    sync.dma_start(
                        out=w_sb[j][:],
                        in_=w_d[:, off * NSL:(off + cpt) * NSL],
                    ).then_inc(s_w[j], 16)
                sync.dma_start(out=hc_d[:], in_=hc[:])._wait_ge(
                    s_done, 1).then_inc(s_out, 16)
                sync.wait_ge(s_out, 16)

            @block.scalar
            def _(scalar):
                src = ifgo if x3 else ps[0:B, :]
                gate_sem, gate_val = (s_dve, 1) if x3 else (s_mm, 1)
                cnew_val = 3 if x3 else 1
                scalar.dma_start(out=sb_ub[:], in_=ub_d[:]).then_inc(s_ub, 16)
                scalar.dma_start(out=sb_c[:], in_=c_d[:]).then_inc(s_c, 16)
                if x3:  # drain u_lo@W_hi rows while PE runs the lo phase
                    scalar.copy(tmpC[:], ps[2 * B:3 * B, :])._wait_ge(
                        s_hi, 1).then_inc(s_act, 1)
                # gate columns ordered [i | f | o | g]
                sig = scalar.activation(gates[:, 0:3 * HS], src[:, 0:3 * HS],
                                        AF.Sigmoid)._wait_ge(gate_sem, gate_val
                                                             ).then_inc(s_act, 1)
                tg = scalar.activation(gates[:, 3 * HS:NSL], src[:, 3 * HS:NSL],
                                       AF.Tanh).then_inc(s_act, 1)
                if x3:
                    tg._wait_ge(s_dve, 2)      # add0b done
                scalar.activation(tnh[:], hc[:, HS:2 * HS],
                                  AF.Tanh)._wait_ge(s_dve, cnew_val
                                                    ).then_inc(s_act, 1)

            @block.tensor
            def _(tensor):
                tensor.wait_ge(s_ub, 16)
                n_hi = KT if x3 else 0
                i = 0
                for j in range(len(chunks)):
                    tensor.wait_ge(s_w[j], 16)
                    for lh, rh, out in plan[j]:
                        mm = tensor.matmul(out, lh, rh, start=(i == 0),
                                           stop=False)
                        i += 1
                        if x3 and i == n_hi:
                            # u_lo@W_hi rows (2B:3B) complete: let ACT copy
                            # them out while the lo phase accumulates 0:B
                            mm.then_inc(s_hi, 1)
                # Bias rank-update last (bias-first via start=True bank-clear
                # measurably breaks on HW despite the documented semantics).
                tensor.matmul(ps[0:B, :], ones, bias,
                              start=False, stop=True).then_inc(s_mm, 1)

            @block.vector
            def _(vector):
                d = 1 if x3 else 0          # s_act offset from the tmpC copy
                vector.wait_ge(s_c, 16)
                if x3:   # fold the u_lo@W_hi rows into the u_hi rows;
                    # split so the sigmoid starts after the first 3 gates
                    vector.wait_ge(s_act, 1)        # tmpC copied
                    vector.tensor_add(ifgo[:, 0:3 * HS], ps[0:B, 0:3 * HS],
                                      tmpC[:, 0:3 * HS])._wait_ge(
                                          s_mm, 1).then_inc(s_dve, 1)
                    vector.tensor_add(ifgo[:, 3 * HS:NSL],
                                      ps[0:B, 3 * HS:NSL],
                                      tmpC[:, 3 * HS:NSL]).then_inc(s_dve, 1)
                vector.tensor_mul(fc[:], gates[:, HS:2 * HS],
                                  sb_c[:])._wait_ge(s_act, 1 + d)  # sigmoid
                vector.tensor_mul(ig[:], gates[:, 0:HS],
                                  gates[:, 3 * HS:NSL])._wait_ge(s_act, 2 + d)
                vector.tensor_add(hc[:, HS:2 * HS], fc[:], ig[:]).then_inc(
                    s_dve, 1)                                        # c_new
                vector.tensor_mul(hc[:, 0:HS], gates[:, 2 * HS:3 * HS],
                                  tnh[:])._wait_ge(s_act, 3 + d).then_inc(
                                      s_done, 1)                     # h_new

    return nc


def _arrange_k(m):
    """[K, n] -> [128, (K//128)*n] (partition-major k-tile layout)."""
    kdim, n = m.shape
    return np.ascontiguousarray(
        m.reshape(kdim // 128, 128, n).transpose(1, 0, 2)).reshape(128, -1)


def _split_hi_lo(a):
    hi = a.astype(ml_dtypes.bfloat16)
    lo = (a - hi.astype(np.float32)).astype(ml_dtypes.bfloat16)
    return hi, lo


def _make_in_maps(inputs, prec):
    npdt = mybir.dt.np(_WDT[prec])
    x3 = prec == 'bf16x3'
    un = 2 * B if x3 else B
    ext = B + NSL

    x = np.asarray(inputs['x'], np.float32)
    h = np.asarray(inputs['h'], np.float32)
    c = np.asarray(inputs['c'], np.float32)
    Wi = np.asarray(inputs['Wi'], np.float32)
    Wh = np.asarray(inputs['Wh'], np.float32)
    bias = (np.asarray(inputs['Wi_b'], np.float32)
            + np.asarray(inputs['Wh_b'], np.float32))

    u = np.concatenate([x, h], axis=1)           # [B, K]
    V = np.concatenate([Wi, Wh], axis=0)         # [K, 4H]

    uT = np.ascontiguousarray(u.T)               # [K, B]
    if x3:
        u_hi, u_lo = _split_hi_lo(uT)
        # per k-tile: [u_hi | zero pad | u_lo] -> [128, KT, 3B]
        zpad = np.zeros((128, KT, B), u_hi.dtype)
        u_flat = np.concatenate(
            [_arrange_k(u_hi).reshape(128, KT, B), zpad,
             _arrange_k(u_lo).reshape(128, KT, B)], axis=2).reshape(128, -1)
    else:
        u_flat = _arrange_k(uT.astype(npdt))

    in_maps = []
    for k in range(N_CORES):
        # gate order [i | f | o | g] (gate blocks 0,1,3,2 of ifgo)
        cols = np.concatenate(
            [np.arange(g * H + k * HS, g * H + (k + 1) * HS) for g in (0, 1, 3, 2)])
        Vk = np.ascontiguousarray(V[:, cols])    # [K, NSL]
        ext_block = np.zeros((128, ext), npdt)
        if x3:
            w_hi, w_lo = _split_hi_lo(Vk)
            w_arr = np.concatenate([_arrange_k(w_hi), _arrange_k(w_lo)], axis=1)
            b_hi, b_lo = _split_hi_lo(bias[cols])
            ext_block[0, B:] = b_hi
            ext_block[1, B:] = b_lo
            ext_block[0:2, :B] = 1.0
        else:
            w_arr = _arrange_k(Vk.astype(npdt))
            ext_block[0, B:] = bias[cols].astype(npdt)
            ext_block[0, :B] = 1.0
        in_maps.append({
            'ub': np.ascontiguousarray(
                np.concatenate([u_flat, ext_block], axis=1)),
            'w': np.ascontiguousarray(w_arr),
            'c': np.ascontiguousarray(c[:, k * HS:(k + 1) * HS]),
        })
    return in_maps


def _run(inputs, prec=None, **spmd_kwargs):
    prec = prec or PREC
    if prec not in _built:
        _built[prec] = _build(prec)
    nc = _built[prec]
    in_maps = _make_in_maps(inputs, prec)
    res = run_bass_kernel_spmd(nc, in_maps, core_ids=list(range(N_CORES)),
                               **spmd_kwargs)
    h_new = np.empty((B, H), np.float32)
    c_new = np.empty((B, H), np.float32)
    for k in range(N_CORES):
        hc = res.results[k]['hc']
        h_new[:, k * HS:(k + 1) * HS] = hc[:, :HS]
        c_new[:, k * HS:(k + 1) * HS] = hc[:, HS:]
    return res, (h_new, c_new)


def kernel(**inputs):
    return _run(inputs)[1]



# revision 7
# speedup vs baseline: 1.8532x; 1.8532x over previous
"""Trainium2 Bass kernel for nn_ContextualLSTMCell_74955769250717.

The reference multiplies the low-rank context perturbations A_i/A_h by 0.0
(faithful to the original model), so the required math reduces exactly to a
plain LSTM cell:

    ifgo  = x @ Wi + Wi_b + h @ Wh + Wh_b            [B, 4H]
    i,f,g,o = gates(ifgo);  c_new = f*c + i*g;  h_new = o*tanh(c_new)

Sharding: tensor-parallel over the hidden dim. Core k owns hidden slice
k*128:(k+1)*128 of every gate (512 of the 4096 gate columns). The host
concatenates the 8 independent slices.

Layout ("transposed" / weight-stationary): each matmul uses the W k-tile
block [128 K-rows, 128 hidden] as the stationary operand and the k-tile of
u.T = [x|h].T [128 K-rows, 16 batch] as the moving operand, accumulating
psum[128 hidden, 16 batch] per gate. This makes every matmul stream only 16
columns, puts the gate/hidden dim on partitions (so the elementwise tail
runs 128-wide), and lets the per-gate bias ride the activation instruction
as a per-partition operand.

Precision: the moving u is fp16 (pre-divided by S=128); every W block is
scaled by S and stored per (gate, k-tile) in either bf16 or fp8-e3m4 (the
NBF map below says how many k-tiles of each gate are bf16). The activation
applies scale=1/S. Gate-wise error sensitivity (g > o > f > i) drives the
map; at the default map the measured rel err is ~6e-3 vs the 2e-2 gate.

Dataflow: one uint8 blob per core ([u | c | bias | W_i | W_f | W_g | W_o])
streamed HBM->SBUF in 5 chunk DMAs in consumption order; PE processes gates
i,f,g,o as chunks land; ACT computes sigmoids/tanh (bias+descale fused);
DVE finishes the cell; the last chunk is a single W_o k-tile so the
post-stream tail is one matmul + one sigmoid + one multiply + output DMA.

Raw Bass (no Tile): this toolchain enforces ONE sync-wait slot per
instruction; with explicit semaphores every join is a chain of standalone
single-wait instructions.
"""

from contextlib import ExitStack

import ml_dtypes
import numpy as np

import concourse.bass as bass
import concourse.mybir as mybir
from concourse.bass_utils import run_bass_kernel_spmd

B, E, H = 16, 1024, 1024
K = E + H                  # combined contraction dim (x and h stacked)
KT = K // 128              # 16 K-tiles of 128
N_CORES = 8
HS = H // N_CORES          # 128 hidden units per core (per gate)
S = 128.0                  # global W/u scale (exact power of two)

AF = mybir.ActivationFunctionType
F32 = mybir.dt.float32
F16 = mybir.dt.float16
BF16 = mybir.dt.bfloat16
E3M4 = mybir.dt.float8e3
U8 = mybir.dt.uint8

# Per-gate count of k-tiles stored in bf16 (the rest are fp8-e3m4).
# Sensitivity order g > o > f > i. bf16 tiles are placed FIRST in each
# gate's k-tile order except for o, where the last tile is e3m4 so the
# final chunk (and the post-stream tail) is as small as possible.
NBF = {'i': 0, 'f': 16, 'g': 16, 'o': 12}

GATES = 'ifgo'             # processing order; also reference column order

# ---- byte layout of the streamed blob (per partition) ----------------------
OFF_U = 0                  # u.T k-tiles, fp16, [128, KT*16]
OFF_C = OFF_U + KT * B * 2         # c.T slice, fp32, [128, 16]
OFF_BIAS = OFF_C + B * 4           # bias, fp32, [128, 4]
OFF_W = OFF_BIAS + 4 * 4           # W blocks, gate-major then k-tile


def _wmap(nbf):
    """Per (gate, kt): (dtype, np_dtype, bytes, blob_offset). bf16 tiles
    first within each gate, except gate o keeps its last tile e3m4."""
    m = {}
    off = OFF_W
    for g in GATES:
        n = nbf[g]
        kinds = ['bf'] * n + ['e3'] * (KT - n)
        if g == 'o' and 0 < n < KT:
            kinds = ['bf'] * n + ['e3'] * (KT - n)   # e3 tail already last
        for kt in range(KT):
            if kinds[kt] == 'bf':
                m[g, kt] = (BF16, ml_dtypes.bfloat16, 256, off)
            else:
                m[g, kt] = (E3M4, ml_dtypes.float8_e3m4, 128, off)
            off += m[g, kt][2]
    return m, off


WMAP, TOT = _wmap(NBF)

# Chunk boundaries (byte ranges of the blob, in stream order):
#   chunk 0: u + c + bias + all of W_i
#   chunk 1: W_f     chunk 2: W_g     chunk 3: W_o k-tiles 0..14
#   chunk 4: W_o k-tile 15 (the tail chunk)
_wi_end = WMAP['f', 0][3]
_wf_end = WMAP['g', 0][3]
_wg_end = WMAP['o', 0][3]
_wo_last = WMAP['o', KT - 1][3]
CHUNKS = [(0, _wi_end), (_wi_end, _wf_end), (_wf_end, _wg_end),
          (_wg_end, _wo_last), (_wo_last, TOT)]
# chunk index whose arrival gates (gate, kt)'s matmul
_CHUNK_OF = {('i', kt): 0 for kt in range(KT)}
_CHUNK_OF.update({('f', kt): 1 for kt in range(KT)})
_CHUNK_OF.update({('g', kt): 2 for kt in range(KT)})
_CHUNK_OF.update({('o', kt): 3 for kt in range(KT - 1)})
_CHUNK_OF['o', KT - 1] = 4

_built = {}


def _build():
    nc = bass.Bass()
    blob_d = nc.dram_tensor("blob", [128, TOT], U8, kind="ExternalInput")
    hc_d = nc.dram_tensor("hc", [128, 2 * B], F32, kind="ExternalOutput")

    with ExitStack() as ctx:
        e = ctx.enter_context
        sb = e(nc.sbuf_tensor("sb", [128, TOT], U8))
        gsb = e(nc.sbuf_tensor("gsb", [128, 4 * B], F32))   # sig_i|sig_f|tanh_g|sig_o
        fc = e(nc.sbuf_tensor("fc", [128, B], F32))
        ig = e(nc.sbuf_tensor("ig", [128, B], F32))
        tnh = e(nc.sbuf_tensor("tnh", [128, B], F32))
        hc = e(nc.sbuf_tensor("hc_sb", [128, 2 * B], F32))  # h | c_new
        ps = e(nc.psum_tensor("ps", [128, 4 * B], F32))

        s_c = [e(nc.semaphore(f"s_c{j}")) for j in range(len(CHUNKS))]
        s_pe = e(nc.semaphore("s_pe"))
        s_act = e(nc.semaphore("s_act"))
        s_dve = e(nc.semaphore("s_dve"))
        s_done = e(nc.semaphore("s_done"))
        s_out = e(nc.semaphore("s_out"))

        u16 = sb[:, OFF_U:OFF_C].bitcast(F16)        # [128, KT*16]
        cT = sb[:, OFF_C:OFF_BIAS].bitcast(F32)      # [128, 16]
        bias4 = sb[:, OFF_BIAS:OFF_W].bitcast(F32)   # [128, 4]

        def wview(g, kt):
            dt, _, nbytes, off = WMAP[g, kt]
            return sb[:, off:off + nbytes].bitcast(dt)

        with nc.Block() as block:

            @block.sync
            def _(sync):
                for j, (lo, hi) in enumerate(CHUNKS):
                    sync.dma_start(out=sb[:, lo:hi],
                                   in_=blob_d[:, lo:hi]).then_inc(s_c[j], 16)
                sync.dma_start(out=hc_d[:], in_=hc[:])._wait_ge(
                    s_done, 2).then_inc(s_out, 16)
                sync.wait_ge(s_out, 16)

            @block.tensor
            def _(tensor):
                # every MM waits on its chunk sem (free once satisfied; the
                # race detector does not credit same-engine program order)
                for jg, g in enumerate(GATES):
                    for kt in range(KT):
                        mm = tensor.matmul(
                            ps[:, jg * B:(jg + 1) * B],
                            wview(g, kt),
                            u16[:, kt * B:(kt + 1) * B],
                            start=(kt == 0), stop=(kt == KT - 1))
                        mm._wait_ge(s_c[_CHUNK_OF[g, kt]], 16)
                        if kt == KT - 1:
                            mm.then_inc(s_pe, 1)

            @block.scalar
            def _(scalar):
                # gsb layout: [sig_i | sig_f | tanh_g | sig_o]
                scalar.activation(gsb[:, 0:B], ps[:, 0:B], AF.Sigmoid,
                                  bias=bias4[:, 0:1], scale=1.0 / S
                                  )._wait_ge(s_pe, 1).then_inc(s_act, 1)
                scalar.activation(gsb[:, B:2 * B], ps[:, B:2 * B], AF.Sigmoid,
                                  bias=bias4[:, 1:2], scale=1.0 / S
                                  )._wait_ge(s_pe, 2).then_inc(s_act, 1)
                scalar.activation(gsb[:, 2 * B:3 * B], ps[:, 2 * B:3 * B],
                                  AF.Tanh, bias=bias4[:, 2:3], scale=1.0 / S
                                  )._wait_ge(s_pe, 3).then_inc(s_act, 1)
                scalar.activation(tnh[:], hc[:, B:2 * B],
                                  AF.Tanh)._wait_ge(s_done, 1).then_inc(s_act, 1)
                scalar.activation(gsb[:, 3 * B:4 * B], ps[:, 3 * B:4 * B],
                                  AF.Sigmoid, bias=bias4[:, 3:4], scale=1.0 / S
                                  )._wait_ge(s_pe, 4).then_inc(s_act, 1)

            @block.vector
            def _(vector):
                vector.tensor_mul(fc[:], gsb[:, B:2 * B],
                                  cT)._wait_ge(s_act, 2).then_inc(s_dve, 1)
                vector.tensor_mul(ig[:], gsb[:, 0:B],
                                  gsb[:, 2 * B:3 * B])._wait_ge(
                                      s_act, 3).then_inc(s_dve, 1)
                vector.tensor_add(hc[:, B:2 * B], fc[:], ig[:])._wait_ge(
                    s_dve, 2).then_inc(s_done, 1)                    # c_new
                vector.tensor_mul(hc[:, 0:B], gsb[:, 3 * B:4 * B],
                                  tnh[:])._wait_ge(s_act, 5).then_inc(
                                      s_done, 1)                     # h_new

    return nc


def _make_in_maps(inputs):
    x = np.asarray(inputs['x'], np.float32)
    h = np.asarray(inputs['h'], np.float32)
    c = np.asarray(inputs['c'], np.float32)
    Wi = np.asarray(inputs['Wi'], np.float32)
    Wh = np.asarray(inputs['Wh'], np.float32)
    bias = (np.asarray(inputs['Wi_b'], np.float32)
            + np.asarray(inputs['Wh_b'], np.float32))

    u = np.concatenate([x, h], axis=1)            # [B, K]
    V = np.concatenate([Wi, Wh], axis=0)          # [K, 4H]

    # u.T in fp16, k-tile-major: [128, KT*B]  (W carries the S scale)
    uT = np.ascontiguousarray(u.T).astype(np.float16)         # [K, B]
    u_arr = np.ascontiguousarray(
        uT.reshape(KT, 128, B).transpose(1, 0, 2)).reshape(128, KT * B)
    u_bytes = u_arr.view(np.uint8)                # [128, KT*B*2]

    in_maps = []
    for k in range(N_CORES):
        hs = slice(k * HS, (k + 1) * HS)
        blob = np.zeros((128, TOT), np.uint8)
        blob[:, OFF_U:OFF_C] = u_bytes
        blob[:, OFF_C:OFF_BIAS] = np.ascontiguousarray(
            c[:, hs].T.astype(np.float32)).view(np.uint8)
        b4 = np.stack([bias[jg * H + k * HS:jg * H + (k + 1) * HS]
                       for jg in range(4)], axis=1)            # [128, 4]
        blob[:, OFF_BIAS:OFF_W] = np.ascontiguousarray(b4).view(np.uint8)
        for jg, g in enumerate(GATES):
            for kt in range(KT):
                _, npdt, nbytes, off = WMAP[g, kt]
                blk = V[kt * 128:(kt + 1) * 128, jg * H + k * HS:
                        jg * H + (k + 1) * HS] * S             # [128, 128]
                blob[:, off:off + nbytes] = np.ascontiguousarray(
                    blk.astype(npdt)).view(np.uint8)
        in_maps.append({'blob': blob})
    return in_maps


def _run(inputs, **spmd_kwargs):
    if 'nc' not in _built:
        _built['nc'] = _build()
    nc = _built['nc']
    in_maps = _make_in_maps(inputs)
    res = run_bass_kernel_spmd(nc, in_maps, core_ids=list(range(N_CORES)),
                               **spmd_kwargs)
    h_new = np.empty((B, H), np.float32)
    c_new = np.empty((B, H), np.float32)
    for k in range(N_CORES):
        hc = res.results[k]['hc']                 # [128, 2B]
        h_new[:, k * HS:(k + 1) * HS] = hc[:, :B].T
        c_new[:, k * HS:(k + 1) * HS] = hc[:, B:].T
    return res, (h_new, c_new)


def kernel(**inputs):
    return _run(inputs)[1]
